# revision 73
# baseline (speedup 1.0000x reference)
"""Trainium2 Bass kernel for an attention block (B=4, C=64, H=W=64).

reference:
    xf = x.reshape(B, C, N)                      # N = H*W = 4096
    qkv = w_qkv @ xf + b_qkv                     # [B, 3C, N]
    q, k, v = split(qkv)
    attn = softmax(q^T k / sqrt(C), axis=-1)     # [B, N, N]
    out = w_proj @ (v @ attn^T) + b_proj + x

Sharding: 8 cores = (batch sample, query half). Each core receives its
sample's tokens ROTATED so its own 2048 queries are always columns
0:2048 (attention is permutation-invariant over keys, so K/V built from
the rotated layout give identical outputs). Each core computes K and V
for its sample plus the attention output for its queries; no
collectives.

The k-side bias is dropped: bk.q is constant over keys, so it cancels
exactly in the softmax. The remaining q/k projections fold into a single
host-side matrix: scores[m, q] = x_m . (A x_q + c) with A = Wk^T Wq and
c = Wk^T b_q (exact by associativity), so the kernel runs one combined
projection instead of separate q and k ones. The score matrix is produced transposed ([keys, queries]) so the
attn @ V contraction needs no transposes, and the softmax denominator
comes out of the same matmul via a ones-column appended to V^T. The
output projection is folded into the V projection weights on the host
(w_vp = w_proj @ w_v), and the division by the softmax denominator is
applied after that projection (it commutes), using a contraction-dim-1
matmul to broadcast the reciprocal row across partitions. The v/proj
biases fold to a single per-channel vector b_eff = w_proj @ b_v + b_proj
because softmax rows sum to one.
"""

import numpy as np

import concourse.bass as bass
import concourse.tile as tile
from concourse import mybir
from concourse.bass_utils import run_bass_kernel_spmd

B, C = 4, 64
N = 4096          # H*W tokens
QH = N // 2       # queries per core
QB = 1024         # scores/exp batch (2 PSUM banks)
NQB = QH // QB
MC = 128          # key chunk = scores partition dim
NMC = N // MC

_F32 = mybir.dt.float32
_F32R = mybir.dt.float32r
_EXP = mybir.ActivationFunctionType.Exp
_BF16 = mybir.dt.bfloat16
_I16 = mybir.dt.int16
# bf16 Schraudolph exp: bf16_bits(exp(s/8)) ~= int16(s * (0.125*2^7*log2 e) + B)
_SCH_A = 0.125 * 128.0 * 1.4426950408889634
_SCH_B = 16249.5
# m-chunks whose exp is computed on the (otherwise idle) vector engine,
# per query block. Block 0 shares its psum slot with the projections, so
# offloaded chunks live only in block 1.
_DVE_EXP_M = ((8, 13, 18, 23, 27, 30), (4, 8, 12, 16, 20, 24, 27, 30))
_ADD = mybir.AluOpType.add


def _r(ap):
    return ap.bitcast(_F32R)


def _split_excess_waits(nc):
    """walrus CoreV3 in this toolchain accepts at most one sync wait per
    instruction; move extras onto NoOps spliced just before it."""
    for f in nc.m.functions:
        for bb in f.blocks:
            new_insts = []
            changed = False
            for inst in bb.instructions:
                si = inst.sync_info
                if si is not None and si.on_wait and len(si.on_wait) > 1:
                    waits = list(si.on_wait)
                    extra, keep = waits[:-1], waits[-1:]
                    for w in extra:
                        nop = mybir.InstNoOp(name=nc.get_next_instruction_name())
                        nop.engine = inst.engine
                        nop.sync_info = mybir.SyncInfo(on_wait=[w], on_update=[])
                        nc.register_instruction(nop)
                        new_insts.append(nop)
                    si.on_wait = keep
                    changed = True
                new_insts.append(inst)
            if changed:
                bb.instructions = new_insts


def build_graph():
    nc = bass.Bass("TRN2", target_bir_lowering=False, debug=False)

    x_ext = nc.declare_dram_parameter("x", [C, N], _F32, isOutput=False)
    # w_qw = (Wk^T Wq)^T ; w_vpT = (w_proj @ w_qkv[2C:3C]).T
    wqkT_ext = nc.declare_dram_parameter("w_qkT", [C, C], _F32, isOutput=False)
    wvpT_ext = nc.declare_dram_parameter("w_vpT", [C, C], _I16, isOutput=False)
    bqk_ext = nc.declare_dram_parameter("b_qk", [C, 1], _F32, isOutput=False)
    beff_ext = nc.declare_dram_parameter("b_eff", [C, 1], _F32, isOutput=False)
    ones_ext = nc.declare_dram_parameter("ones", [MC, C, 1], _F32, isOutput=False)
    ones16_ext = nc.declare_dram_parameter("ones16", [MC, NMC, 1], _I16, isOutput=False)
    out_ext = nc.declare_dram_parameter("out", [C, QH], _F32, isOutput=True)

    with (
        nc.allow_low_precision(reason="float32r is 32-bit storage"),
        tile.TileContext(nc) as tc,
        tc.tile_pool(name="consts", bufs=1) as consts,
        # PSUM budget (8 banks): s 2x[128,1024]=4, av 1x[65,1024]=2,
        # pj 1x[128,1024]=2. Pre-loop projections borrow the s ring; in-loop
        # projections, the epilogue broadcast, and the DVE-exp score chunks
        # share the pj slot so the main scores ring never waits on the DVE.
        tc.tile_pool(name="spool", bufs=2, space="PSUM") as spool,
        tc.tile_pool(name="avpool", bufs=1, space="PSUM") as avpool,
        tc.tile_pool(name="pjpool", bufs=1, space="PSUM") as pjpool,
        tc.tile_pool(name="ebuf", bufs=5) as ebuf,
        tc.tile_pool(name="obuf", bufs=4) as obuf,
    ):
        X = consts.tile([C, N], _F32R, tag="x")
        WQK = consts.tile([C, C], _F32R, tag="wqk")
        WVP = consts.tile([C, C], _BF16, tag="wvp")
        XB = consts.tile([C, N], _BF16, tag="xb")
        BQK = consts.tile([C, 1], _F32, tag="bqk")
        BEFF = consts.tile([C, 1], _F32, tag="beff")
        ONES1 = consts.tile([1, C], _F32R, tag="ones1")
        QW = consts.tile([C, QH], _F32R, tag="qw")
        VT = consts.tile([MC, NMC * 65], _BF16, tag="vt")
        VT3 = VT.rearrange("p (n c) -> p n c", c=65)

        # weights/biases lead each DMA queue, then x chunks interleave
        # across the gpsimd and sync queues
        def dma_x(eng, j):
            eng.dma_start(
                out=X[:, j * 512 : (j + 1) * 512],
                in_=_r(x_ext[:, j * 512 : (j + 1) * 512]),
            )

        dma_x(nc.gpsimd, 0)
        nc.sync.dma_start(out=WQK, in_=_r(wqkT_ext[:, :]))
        dma_x(nc.sync, 1)
        nc.sync.dma_start(out=BQK, in_=bqk_ext[:, :])
        nc.sync.dma_start(out=VT3[:, :, 64:65].bitcast(_I16), in_=ones16_ext[:, :, :])
        dma_x(nc.gpsimd, 2)
        nc.sync.dma_start(out=WVP.bitcast(_I16), in_=wvpT_ext[:, :])
        dma_x(nc.sync, 3)
        dma_x(nc.gpsimd, 4)
        nc.sync.dma_start(out=ONES1, in_=_r(ones_ext[0:1, :, 0]))
        nc.sync.dma_start(out=BEFF, in_=beff_ext[:, :])
        dma_x(nc.sync, 5)
        dma_x(nc.gpsimd, 6)
        dma_x(nc.sync, 7)
        # preload the Exp table (1283ns) while DMAs are in flight
        WARM = consts.tile([1, 1], _F32, tag="warm")
        nc.vector.memset(WARM, 0.0)
        nc.scalar.activation(WARM, WARM, _EXP, bias=0.0, scale=1.0)

        # ---- projections, emitted just-in-time inside the attention loop
        # so no engine queue stalls on a not-yet-DMAed x chunk ----
        def emit_qw(j, pool):
            # fused projection: qw = A x_q + c, scores = x^T qw
            lo, hi = j * 512, (j + 1) * 512
            ps = pool.tile([C, 512], _F32, tag="pj" if pool is pjpool else "s")
            nc.tensor.matmul(ps, WQK, X[:, lo:hi], start=True, stop=True)
            nc.vector.tensor_scalar_add(QW[:, lo:hi], ps, BQK)

        def emit_vp(g, pool):
            # projected v, transposed, 4 chunks per psum tile + 1 strided
            # copy; bf16 so the 64-wide matmuls run at 1 cycle/row
            lo, hi = g * 512, (g + 1) * 512
            nc.vector.tensor_copy(XB[:, lo:hi], X[:, lo:hi])
            ps = pool.tile([MC, 4, C], _F32, tag="pj" if pool is pjpool else "s")
            for i in range(4):
                m = g * 4 + i
                nc.tensor.matmul(
                    ps[:, i, :],
                    XB[:, m * MC : (m + 1) * MC],
                    WVP,
                    start=True,
                    stop=True,
                )
            nc.vector.tensor_copy(VT3[:, g * 4 : (g + 1) * 4, 0:C], ps)

        emit_qw(0, spool)
        emit_qw(1, spool)
        emit_vp(0, spool)
        hooks = {}
        hooks.setdefault(1, []).append((emit_qw, 2))
        hooks.setdefault(2, []).append((emit_qw, 3))
        for g in range(1, 8):
            hooks.setdefault(3 * g + 1, []).append((emit_vp, g))

        # ---- attention ----
        def emit_ucopies(pav, esz, ucopy_engine):
            # read the finished accumulator out of psum right away so its
            # slot is released before the next block claims it
            us = []
            for h in range(0, QB, esz):
                U = obuf.tile([C + 1, esz], _F32, tag="u")
                ucopy_engine(U, pav[:, h : h + esz])
                us.append(U)
            return us

        def epilogue(qb, us, esz, stt_engine):
            q0 = qb * QB
            for h in range(0, QB, esz):
                U = us[h // esz]
                R1 = obuf.tile([1, esz], _F32R, tag="r1")
                nc.vector.reciprocal(R1, U[C : C + 1, :])
                pb = pjpool.tile([C, esz], _F32, tag="pj")
                nc.tensor.matmul(pb, ONES1, R1, start=True, stop=True)
                UN = obuf.tile([C, esz], _F32, tag="un")
                nc.vector.tensor_mul(UN, U[0:C, :], pb)
                O = obuf.tile([C, esz], _F32, tag="o")
                if stt_engine is None:
                    nc.gpsimd.tensor_scalar_add(O, UN, BEFF)
                    nc.gpsimd.tensor_add(O, O, X[:, q0 + h : q0 + h + esz])
                else:
                    stt_engine(
                        out=O,
                        in0=UN,
                        scalar=BEFF,
                        in1=X[:, q0 + h : q0 + h + esz],
                        op0=_ADD,
                        op1=_ADD,
                    )
                nc.sync.dma_start(out=out_ext[:, q0 + h : q0 + h + esz], in_=O)

        pending = None  # (qb, U tiles) whose epilogue is emitted into the next block
        prev_av = None  # AV rides one stage behind scores (even across blocks)
        for qb in range(NQB):
            q0 = qb * QB
            pav = None  # allocated after the previous block's tail AV is flushed

            def emit_av(m, E, pav_):
                for h in (0, 512):
                    nc.tensor.matmul(
                        pav_[:, h : h + 512],
                        VT[:, m * 65 : (m + 1) * 65],
                        E[:, h : h + 512],
                        start=(m == 0),
                        stop=(m == NMC - 1),
                    )

            dve_m = _DVE_EXP_M[qb]
            for m in range(NMC):
                if m in dve_m:
                    pss = pjpool.tile([MC, QB], _F32, tag="pj")
                else:
                    pss = spool.tile([MC, QB], _F32, tag="s")
                for h in (0, 512):
                    nc.tensor.matmul(
                        pss[:, h : h + 512],
                        X[:, m * MC : (m + 1) * MC],
                        QW[:, q0 + h : q0 + h + 512],
                        start=True,
                        stop=True,
                    )
                if prev_av is not None:
                    pm, pE, ppav = prev_av
                    emit_av(pm, pE, ppav)
                    prev_av = None
                    if pm == NMC - 1:
                        # previous block's tail: drain its accumulator now
                        pending = (qb - 1, emit_ucopies(ppav, 512, nc.vector.tensor_copy))
                if pav is None:
                    pav = avpool.tile([C + 1, QB], _F32, tag="av")
                if m in dve_m:
                    Ei = ebuf.tile([MC, QB], _I16, tag="e")
                    nc.vector.tensor_scalar(
                        out=Ei,
                        in0=pss,
                        scalar1=_SCH_A,
                        scalar2=_SCH_B,
                        op0=mybir.AluOpType.mult,
                        op1=_ADD,
                    )
                    E = Ei.bitcast(_BF16)
                else:
                    E = ebuf.tile([MC, QB], _BF16, tag="e")
                    nc.scalar.activation(E, pss, _EXP, bias=0.0, scale=0.125)
                prev_av = (m, E, pav)
                if qb == 0:
                    for fn, arg in hooks.get(m, ()):
                        fn(arg, pjpool)
                if m == 6 and pending is not None:
                    # previous block's epilogue rides behind this block's
                    # first few iterations in every engine queue
                    epilogue(
                        *pending,
                        esz=512,
                        stt_engine=nc.vector.scalar_tensor_tensor,
                    )
                    pending = None
        # last block's tail AV + epilogue: fine-grained, with the
        # psum->sbuf copy on the (now idle) scalar engine
        pm, pE, ppav = prev_av
        emit_av(pm, pE, ppav)
        pending = (NQB - 1, emit_ucopies(ppav, 256, nc.scalar.copy))
        epilogue(*pending, esz=256, stt_engine=None)

    _split_excess_waits(nc)
    return nc


_GRAPH_CACHE = {}


def _get_graph():
    if "nc" not in _GRAPH_CACHE:
        _GRAPH_CACHE["nc"] = build_graph()
    return _GRAPH_CACHE["nc"]


_ONES = np.ones((MC, C, 1), dtype=np.float32)
_ONES16 = np.full((MC, NMC, 1), 0x3F80, dtype=np.int16)  # bf16 bits of 1.0


def make_in_maps(x, w_qkv, b_qkv, w_proj, b_proj):
    xf = np.ascontiguousarray(np.asarray(x, dtype=np.float32).reshape(B, C, N))
    w_qkv = np.asarray(w_qkv, dtype=np.float32)
    b_qkv = np.asarray(b_qkv, dtype=np.float32)
    w_proj = np.asarray(w_proj, dtype=np.float32)
    b_proj = np.asarray(b_proj, dtype=np.float32)

    # scores = x_m . (A x_q + c): A = Wk^T Wq, c = Wk^T b_q (weight prep)
    A = w_qkv[C : 2 * C].T @ w_qkv[0:C]
    w_qkT = np.ascontiguousarray(A.T.astype(np.float32))
    # fold the output projection into the v projection (weight prep):
    # w_proj @ (w_v @ x) == (w_proj @ w_v) @ x
    w_vpT_f = np.ascontiguousarray((w_proj @ w_qkv[2 * C :]).T.astype(np.float32))
    u = w_vpT_f.view(np.uint32)
    w_vpT = ((u + 0x7FFF + ((u >> 16) & 1)) >> 16).astype(np.uint16).view(np.int16)
    b_qk = (w_qkv[C : 2 * C].T @ b_qkv[0:C]).reshape(C, 1).astype(np.float32)
    # v/proj biases fold to one vector because softmax rows sum to 1
    b_eff = (w_proj @ b_qkv[2 * C :] + b_proj).reshape(C, 1).astype(np.float32)

    in_maps = []
    for core in range(8):
        b, h = divmod(core, 2)
        # rotate tokens so this core's queries are columns 0:QH
        xr = np.ascontiguousarray(np.roll(xf[b], -h * QH, axis=1))
        in_maps.append(
            {
                "x": xr,
                "w_qkT": w_qkT,
                "w_vpT": w_vpT,
                "b_qk": b_qk,
                "b_eff": b_eff,
                "ones": _ONES,
                "ones16": _ONES16,
            }
        )
    return in_maps


def kernel(x, w_qkv, b_qkv, w_proj, b_proj):
    x = np.asarray(x)
    nc = _get_graph()
    in_maps = make_in_maps(x, w_qkv, b_qkv, w_proj, b_proj)
    res = run_bass_kernel_spmd(nc, in_maps, core_ids=list(range(8)))
    out = np.empty((B, C, N), dtype=np.float32)
    for core in range(8):
        b, h = divmod(core, 2)
        out[b][:, h * QH : (h + 1) * QH] = res.results[core]["out"]
    return out.reshape(x.shape).astype(np.float32)


# revision 74
# speedup vs baseline: 1.0099x; 1.0099x over previous
"""Trainium2 Bass kernel for an attention block (B=4, C=64, H=W=64).

reference:
    xf = x.reshape(B, C, N)                      # N = H*W = 4096
    qkv = w_qkv @ xf + b_qkv                     # [B, 3C, N]
    q, k, v = split(qkv)
    attn = softmax(q^T k / sqrt(C), axis=-1)     # [B, N, N]
    out = w_proj @ (v @ attn^T) + b_proj + x

Sharding: 8 cores = (batch sample, query half). Each core receives its
sample's tokens ROTATED so its own 2048 queries are always columns
0:2048 (attention is permutation-invariant over keys, so K/V built from
the rotated layout give identical outputs). Each core computes K and V
for its sample plus the attention output for its queries; no
collectives.

The k-side bias is dropped: bk.q is constant over keys, so it cancels
exactly in the softmax. The remaining q/k projections fold into a single
host-side matrix: scores[m, q] = x_m . (A x_q + c) with A = Wk^T Wq and
c = Wk^T b_q (exact by associativity), so the kernel runs one combined
projection instead of separate q and k ones. The score matrix is produced transposed ([keys, queries]) so the
attn @ V contraction needs no transposes, and the softmax denominator
comes out of the same matmul via a ones-column appended to V^T. The
output projection is folded into the V projection weights on the host
(w_vp = w_proj @ w_v), and the division by the softmax denominator is
applied after that projection (it commutes), using a contraction-dim-1
matmul to broadcast the reciprocal row across partitions. The v/proj
biases fold to a single per-channel vector b_eff = w_proj @ b_v + b_proj
because softmax rows sum to one.
"""

import numpy as np

import concourse.bass as bass
import concourse.tile as tile
from concourse import mybir
from concourse.bass_utils import run_bass_kernel_spmd

B, C = 4, 64
N = 4096          # H*W tokens
QH = N // 2       # queries per core
QB = 1024         # scores/exp batch (2 PSUM banks)
NQB = QH // QB
MC = 128          # key chunk = scores partition dim
NMC = N // MC

_F32 = mybir.dt.float32
_F32R = mybir.dt.float32r
_EXP = mybir.ActivationFunctionType.Exp
_BF16 = mybir.dt.bfloat16
_I16 = mybir.dt.int16
# bf16 Schraudolph exp: bf16_bits(exp(s/8)) ~= int16(s * (0.125*2^7*log2 e) + B)
_SCH_A = 0.125 * 128.0 * 1.4426950408889634
_SCH_B = 16249.5
# m-chunks whose exp is computed on the (otherwise idle) vector engine,
# per query block. Block 0 shares its psum slot with the projections, so
# offloaded chunks live only in block 1.
_DVE_EXP_M = ((8, 12, 16, 20, 24, 27, 30), (4, 8, 12, 16, 20, 24, 27, 30))
_ADD = mybir.AluOpType.add


def _r(ap):
    return ap.bitcast(_F32R)


def _split_excess_waits(nc):
    """walrus CoreV3 in this toolchain accepts at most one sync wait per
    instruction; move extras onto NoOps spliced just before it."""
    for f in nc.m.functions:
        for bb in f.blocks:
            new_insts = []
            changed = False
            for inst in bb.instructions:
                si = inst.sync_info
                if si is not None and si.on_wait and len(si.on_wait) > 1:
                    waits = list(si.on_wait)
                    extra, keep = waits[:-1], waits[-1:]
                    for w in extra:
                        nop = mybir.InstNoOp(name=nc.get_next_instruction_name())
                        nop.engine = inst.engine
                        nop.sync_info = mybir.SyncInfo(on_wait=[w], on_update=[])
                        nc.register_instruction(nop)
                        new_insts.append(nop)
                    si.on_wait = keep
                    changed = True
                new_insts.append(inst)
            if changed:
                bb.instructions = new_insts


def build_graph():
    nc = bass.Bass("TRN2", target_bir_lowering=False, debug=False)

    x_ext = nc.declare_dram_parameter("x", [C, N], _F32, isOutput=False)
    # w_qw = (Wk^T Wq)^T ; w_vpT = (w_proj @ w_qkv[2C:3C]).T
    wqkT_ext = nc.declare_dram_parameter("w_qkT", [C, C], _F32, isOutput=False)
    wvpT_ext = nc.declare_dram_parameter("w_vpT", [C, C], _I16, isOutput=False)
    bqk_ext = nc.declare_dram_parameter("b_qk", [C, 1], _F32, isOutput=False)
    beff_ext = nc.declare_dram_parameter("b_eff", [C, 1], _F32, isOutput=False)
    ones_ext = nc.declare_dram_parameter("ones", [MC, C, 1], _F32, isOutput=False)
    ones16_ext = nc.declare_dram_parameter("ones16", [MC, NMC, 1], _I16, isOutput=False)
    out_ext = nc.declare_dram_parameter("out", [C, QH], _F32, isOutput=True)

    with (
        nc.allow_low_precision(reason="float32r is 32-bit storage"),
        tile.TileContext(nc) as tc,
        tc.tile_pool(name="consts", bufs=1) as consts,
        # PSUM budget (8 banks): s 2x[128,1024]=4, av 1x[65,1024]=2,
        # pj 1x[128,1024]=2. Pre-loop projections borrow the s ring; in-loop
        # projections, the epilogue broadcast, and the DVE-exp score chunks
        # share the pj slot so the main scores ring never waits on the DVE.
        tc.tile_pool(name="spool", bufs=2, space="PSUM") as spool,
        tc.tile_pool(name="avpool", bufs=1, space="PSUM") as avpool,
        tc.tile_pool(name="pjpool", bufs=1, space="PSUM") as pjpool,
        tc.tile_pool(name="ebuf", bufs=5) as ebuf,
        tc.tile_pool(name="obuf", bufs=4) as obuf,
    ):
        X = consts.tile([C, N], _F32R, tag="x")
        WQK = consts.tile([C, C], _F32R, tag="wqk")
        WVP = consts.tile([C, C], _BF16, tag="wvp")
        XB = consts.tile([C, N], _BF16, tag="xb")
        BQK = consts.tile([C, 1], _F32, tag="bqk")
        BEFF = consts.tile([C, 1], _F32, tag="beff")
        ONES1 = consts.tile([1, C], _F32R, tag="ones1")
        QW = consts.tile([C, QH], _F32R, tag="qw")
        VT = consts.tile([MC, NMC * 65], _BF16, tag="vt")
        VT3 = VT.rearrange("p (n c) -> p n c", c=65)

        # weights/biases lead each DMA queue, then x chunks interleave
        # across the gpsimd and sync queues
        def dma_x(eng, j):
            eng.dma_start(
                out=X[:, j * 512 : (j + 1) * 512],
                in_=_r(x_ext[:, j * 512 : (j + 1) * 512]),
            )

        dma_x(nc.gpsimd, 0)
        nc.sync.dma_start(out=WQK, in_=_r(wqkT_ext[:, :]))
        dma_x(nc.sync, 1)
        nc.sync.dma_start(out=BQK, in_=bqk_ext[:, :])
        nc.sync.dma_start(out=VT3[:, :, 64:65].bitcast(_I16), in_=ones16_ext[:, :, :])
        dma_x(nc.gpsimd, 2)
        nc.sync.dma_start(out=WVP.bitcast(_I16), in_=wvpT_ext[:, :])
        dma_x(nc.sync, 3)
        dma_x(nc.gpsimd, 4)
        nc.sync.dma_start(out=ONES1, in_=_r(ones_ext[0:1, :, 0]))
        nc.sync.dma_start(out=BEFF, in_=beff_ext[:, :])
        dma_x(nc.sync, 5)
        dma_x(nc.gpsimd, 6)
        dma_x(nc.sync, 7)
        # preload the Exp table (1283ns) while DMAs are in flight
        WARM = consts.tile([1, 1], _F32, tag="warm")
        nc.vector.memset(WARM, 0.0)
        nc.scalar.activation(WARM, WARM, _EXP, bias=0.0, scale=1.0)

        # ---- projections, emitted just-in-time inside the attention loop
        # so no engine queue stalls on a not-yet-DMAed x chunk ----
        def emit_qw(j, pool):
            # fused projection: qw = A x_q + c, scores = x^T qw
            lo, hi = j * 512, (j + 1) * 512
            ps = pool.tile([C, 512], _F32, tag="pj" if pool is pjpool else "s")
            nc.tensor.matmul(ps, WQK, X[:, lo:hi], start=True, stop=True)
            nc.vector.tensor_scalar_add(QW[:, lo:hi], ps, BQK)

        def emit_vp(g, pool):
            # projected v, transposed, 4 chunks per psum tile + 1 strided
            # copy; bf16 so the 64-wide matmuls run at 1 cycle/row
            lo, hi = g * 512, (g + 1) * 512
            nc.vector.tensor_copy(XB[:, lo:hi], X[:, lo:hi])
            ps = pool.tile([MC, 4, C], _F32, tag="pj" if pool is pjpool else "s")
            for i in range(4):
                m = g * 4 + i
                nc.tensor.matmul(
                    ps[:, i, :],
                    XB[:, m * MC : (m + 1) * MC],
                    WVP,
                    start=True,
                    stop=True,
                )
            nc.vector.tensor_copy(VT3[:, g * 4 : (g + 1) * 4, 0:C], ps)

        emit_qw(0, spool)
        emit_qw(1, spool)
        emit_vp(0, spool)
        hooks = {}
        hooks.setdefault(1, []).append((emit_qw, 2))
        hooks.setdefault(2, []).append((emit_qw, 3))
        for g in range(1, 8):
            hooks.setdefault(3 * g + 1, []).append((emit_vp, g))

        # ---- attention ----
        def emit_ucopies(pav, esz, ucopy_engine):
            # read the finished accumulator out of psum right away so its
            # slot is released before the next block claims it
            us = []
            for h in range(0, QB, esz):
                U = obuf.tile([C + 1, esz], _F32, tag="u")
                ucopy_engine(U, pav[:, h : h + esz])
                us.append(U)
            return us

        def epilogue(qb, us, esz, stt_engine):
            q0 = qb * QB
            for h in range(0, QB, esz):
                U = us[h // esz]
                R1 = obuf.tile([1, esz], _F32R, tag="r1")
                nc.vector.reciprocal(R1, U[C : C + 1, :])
                pb = pjpool.tile([C, esz], _F32, tag="pj")
                nc.tensor.matmul(pb, ONES1, R1, start=True, stop=True)
                UN = obuf.tile([C, esz], _F32, tag="un")
                nc.vector.tensor_mul(UN, U[0:C, :], pb)
                O = obuf.tile([C, esz], _F32, tag="o")
                if stt_engine is None:
                    nc.gpsimd.tensor_scalar_add(O, UN, BEFF)
                    nc.gpsimd.tensor_add(O, O, X[:, q0 + h : q0 + h + esz])
                else:
                    stt_engine(
                        out=O,
                        in0=UN,
                        scalar=BEFF,
                        in1=X[:, q0 + h : q0 + h + esz],
                        op0=_ADD,
                        op1=_ADD,
                    )
                nc.sync.dma_start(out=out_ext[:, q0 + h : q0 + h + esz], in_=O)

        pending = None  # (qb, U tiles) whose epilogue is emitted into the next block
        prev_av = None  # AV rides one stage behind scores (even across blocks)
        for qb in range(NQB):
            q0 = qb * QB
            pav = None  # allocated after the previous block's tail AV is flushed

            def emit_av(m, E, pav_):
                for h in (0, 512):
                    nc.tensor.matmul(
                        pav_[:, h : h + 512],
                        VT[:, m * 65 : (m + 1) * 65],
                        E[:, h : h + 512],
                        start=(m == 0),
                        stop=(m == NMC - 1),
                    )

            dve_m = _DVE_EXP_M[qb]
            for m in range(NMC):
                if m in dve_m:
                    pss = pjpool.tile([MC, QB], _F32, tag="pj")
                else:
                    pss = spool.tile([MC, QB], _F32, tag="s")
                for h in (0, 512):
                    nc.tensor.matmul(
                        pss[:, h : h + 512],
                        X[:, m * MC : (m + 1) * MC],
                        QW[:, q0 + h : q0 + h + 512],
                        start=True,
                        stop=True,
                    )
                if prev_av is not None:
                    pm, pE, ppav = prev_av
                    emit_av(pm, pE, ppav)
                    prev_av = None
                    if pm == NMC - 1:
                        # previous block's tail: drain its accumulator now
                        pending = (qb - 1, emit_ucopies(ppav, 512, nc.vector.tensor_copy))
                if pav is None:
                    pav = avpool.tile([C + 1, QB], _F32, tag="av")
                if m in dve_m:
                    Ei = ebuf.tile([MC, QB], _I16, tag="e")
                    nc.vector.tensor_scalar(
                        out=Ei,
                        in0=pss,
                        scalar1=_SCH_A,
                        scalar2=_SCH_B,
                        op0=mybir.AluOpType.mult,
                        op1=_ADD,
                    )
                    E = Ei.bitcast(_BF16)
                else:
                    E = ebuf.tile([MC, QB], _BF16, tag="e")
                    nc.scalar.activation(E, pss, _EXP, bias=0.0, scale=0.125)
                prev_av = (m, E, pav)
                if qb == 0:
                    for fn, arg in hooks.get(m, ()):
                        fn(arg, pjpool)
                if m == 6 and pending is not None:
                    # previous block's epilogue rides behind this block's
                    # first few iterations in every engine queue
                    epilogue(
                        *pending,
                        esz=512,
                        stt_engine=nc.vector.scalar_tensor_tensor,
                    )
                    pending = None
        # last block's tail AV + epilogue: fine-grained, with the
        # psum->sbuf copy on the (now idle) scalar engine
        pm, pE, ppav = prev_av
        emit_av(pm, pE, ppav)
        pending = (NQB - 1, emit_ucopies(ppav, 256, nc.scalar.copy))
        epilogue(*pending, esz=256, stt_engine=None)

    _split_excess_waits(nc)
    return nc


_GRAPH_CACHE = {}


def _get_graph():
    if "nc" not in _GRAPH_CACHE:
        _GRAPH_CACHE["nc"] = build_graph()
    return _GRAPH_CACHE["nc"]


_ONES = np.ones((MC, C, 1), dtype=np.float32)
_ONES16 = np.full((MC, NMC, 1), 0x3F80, dtype=np.int16)  # bf16 bits of 1.0


def make_in_maps(x, w_qkv, b_qkv, w_proj, b_proj):
    xf = np.ascontiguousarray(np.asarray(x, dtype=np.float32).reshape(B, C, N))
    w_qkv = np.asarray(w_qkv, dtype=np.float32)
    b_qkv = np.asarray(b_qkv, dtype=np.float32)
    w_proj = np.asarray(w_proj, dtype=np.float32)
    b_proj = np.asarray(b_proj, dtype=np.float32)

    # scores = x_m . (A x_q + c): A = Wk^T Wq, c = Wk^T b_q (weight prep)
    A = w_qkv[C : 2 * C].T @ w_qkv[0:C]
    w_qkT = np.ascontiguousarray(A.T.astype(np.float32))
    # fold the output projection into the v projection (weight prep):
    # w_proj @ (w_v @ x) == (w_proj @ w_v) @ x
    w_vpT_f = np.ascontiguousarray((w_proj @ w_qkv[2 * C :]).T.astype(np.float32))
    u = w_vpT_f.view(np.uint32)
    w_vpT = ((u + 0x7FFF + ((u >> 16) & 1)) >> 16).astype(np.uint16).view(np.int16)
    b_qk = (w_qkv[C : 2 * C].T @ b_qkv[0:C]).reshape(C, 1).astype(np.float32)
    # v/proj biases fold to one vector because softmax rows sum to 1
    b_eff = (w_proj @ b_qkv[2 * C :] + b_proj).reshape(C, 1).astype(np.float32)

    in_maps = []
    for core in range(8):
        b, h = divmod(core, 2)
        # rotate tokens so this core's queries are columns 0:QH
        xr = np.ascontiguousarray(np.roll(xf[b], -h * QH, axis=1))
        in_maps.append(
            {
                "x": xr,
                "w_qkT": w_qkT,
                "w_vpT": w_vpT,
                "b_qk": b_qk,
                "b_eff": b_eff,
                "ones": _ONES,
                "ones16": _ONES16,
            }
        )
    return in_maps


def kernel(x, w_qkv, b_qkv, w_proj, b_proj):
    x = np.asarray(x)
    nc = _get_graph()
    in_maps = make_in_maps(x, w_qkv, b_qkv, w_proj, b_proj)
    res = run_bass_kernel_spmd(nc, in_maps, core_ids=list(range(8)))
    out = np.empty((B, C, N), dtype=np.float32)
    for core in range(8):
        b, h = divmod(core, 2)
        out[b][:, h * QH : (h + 1) * QH] = res.results[core]["out"]
    return out.reshape(x.shape).astype(np.float32)
